# revision 15
# baseline (speedup 1.0000x reference)
"""Trainium2 Bass kernel v4: Conv3d(3,24,k=3,VALID) -> min depth -> softmax(ch).

Input  x: [16,3,32,128,128] f32, conv_weight [24,3,3,3,3], conv_bias [24].
Output: [16,24,1,126,126] f32.

v4 strategy (per core; batch-sharded 2 samples/core over 8 cores):
 - kw folded into K: partition k = g*64 + kw*21 + ci*7 + hl (63 used rows per
   64-row group g = sample). Host pre-shifts each kw copy's w-window so the
   matmul rhs is one contiguous [dg,128] run. Only kd remains as 3 PSUM
   accumulation passes (vs 9 in v3) -> 3x fewer streamed columns.
 - 2 concurrent 64-row PE tile groups (one per sample), M = 120 = hoff*24+co.
 - 26 h-blocks of 5 output rows per sample; per block 4 rounds of 2 PSUM
   banks, chunks (4d,128w)x7 + (2d,128w)x1 = 30 depth layers.
 - Epilogue: monotone-exp trick: exp(min y) = min exp(y). ACT streams most
   PSUM banks through exp(-z+bias)->fp16 SBUF; DVE direct-max-reduces the
   rest from PSUM; DVE fp16 TT-min tree (2x mode) collapses depth.
 - Softmax: PE ones-matmul sums channels, DVE reciprocal, PE broadcast
   matmul, DVE multiply. No 128x128 transpose.
"""
import sys

sys.path.insert(0, "/opt/trn_rl_repo")

import numpy as np

# Problem constants
N_TOT, CI, D, H, W = 16, 3, 32, 128, 128
CO = 24
DO, HO, WO = 30, 126, 126
NCORES = 8
NPC = N_TOT // NCORES  # samples per core = 2

NBLK = 26            # h-blocks of 5 output rows
XBUFS = 10           # per-block input tiles in the prefetch rotation
WP = 128             # stored w extent per partition (pre-shifted per kw)
BLK_ELEM = D * WP    # 4096 elements per (partition, block)
# four 4-bank PSUM rounds per block: depth chunks (d0, dg).
# Rounds 0-2 drain via ACT exp; round 3 via DVE direct max-reduce.
# Sizes [7,8,8,7]: round 0 is the round gated by ACT's last queued exp
# (the unavoidable ACT-idle hole), so keeping it small shortens the
# per-block critical chain; the 7-layer round 3 keeps the DVE reduce
# within budget.
ROUNDS4 = [((0, 4), (4, 3)), ((7, 4), (11, 4)),
           ((15, 4), (19, 4)), ((23, 4), (27, 3))]
EXPL0 = (0, 7, 15)   # eblk layer base per exp round

_cache = {}


def _build_program():
    import concourse.bass as bass
    import concourse.mybir as mybir
    from concourse import bacc, tile

    dt = mybir.dt
    f32 = dt.float32
    f16 = dt.float16
    AX = mybir.AxisListType
    ALU = mybir.AluOpType
    ACT_F = mybir.ActivationFunctionType

    nc = bacc.Bacc("TRN2", target_bir_lowering=False, debug=False)

    xs = nc.dram_tensor("xs", [NBLK, 128, BLK_ELEM], f16,
                        kind="ExternalInput")
    wt = nc.dram_tensor("wt", [128, 3 * 128], f16, kind="ExternalInput")
    # per block: [kind 0] min_{d<24} exp(conv) and [kind 1] max_{d>=24} z;
    # host combines as min(e, exp(-maxz)) then softmaxes. [n,block,kind,p,w]
    eout = nc.dram_tensor("eout", [NPC, NBLK, 2, 120, WO], f16,
                          kind="ExternalOutput")

    with tile.TileContext(nc) as tc:
        with (
            tc.tile_pool(name="const", bufs=1) as constp,
            tc.tile_pool(name="xblk", bufs=XBUFS) as xblkp,
            tc.tile_pool(name="expp", bufs=2) as expp,
            tc.tile_pool(name="soft", bufs=2) as softp,
            tc.tile_pool(name="ps", bufs=1, space="PSUM") as psp,
        ):
            wt_t = constp.tile([128, 3 * 128], f16)
            nc.sync.dma_start(wt_t[:], wt[:])

            pending = []  # deferred tree closures (lagged one block)

            def tree_block(B, ev, emxv):
                def emit():
                    # joint min tree over 24 exp layers for both groups
                    s1 = softp.tile([128, 2 * 12 * 128], f16, tag="s1")
                    s1v = s1[:].rearrange("p (g l w) -> p g l w",
                                          g=2, w=128)
                    # 23 exp layers -> 12 (layer 11 overlaps; min idempotent)
                    nc.vector.tensor_tensor(
                        s1v[:, :, 0:12, :], ev[:, :, 0:12, :],
                        ev[:, :, 11:23, :], op=ALU.min)
                    s2 = softp.tile([128, 2 * 6 * 128], f16, tag="s2")
                    s2v = s2[:].rearrange("p (g l w) -> p g l w",
                                          g=2, w=128)
                    nc.vector.tensor_tensor(
                        s2v[:, :, 0:6, :], s1v[:, :, 0:6, :],
                        s1v[:, :, 6:12, :], op=ALU.min)
                    s3 = softp.tile([128, 2 * 3 * 128], f16, tag="s3")
                    s3v = s3[:].rearrange("p (g l w) -> p g l w",
                                          g=2, w=128)
                    nc.vector.tensor_tensor(
                        s3v[:, :, 0:3, :], s2v[:, :, 0:3, :],
                        s2v[:, :, 3:6, :], op=ALU.min)
                    s4 = softp.tile([128, 2 * 2 * 128], f16, tag="s4")
                    s4v = s4[:].rearrange("p (g l w) -> p g l w",
                                          g=2, w=128)
                    nc.vector.tensor_tensor(
                        s4v[:, :, 0:2, :], s3v[:, :, 0:2, :],
                        s3v[:, :, 1:3, :], op=ALU.min)
                    e16 = softp.tile([128, 2 * 128], f16, tag="e16")
                    e16v = e16[:].rearrange("p (g w) -> p g w", w=128)
                    nc.vector.tensor_tensor(
                        e16v[:, :, :], s4v[:, :, 0, :], s4v[:, :, 1, :],
                        op=ALU.min)
                    for g in range(NPC):
                        nc.sync.dma_start(eout[g, B, 0],
                                          e16v[0:120, g, 0:WO])
                        nc.sync.dma_start(eout[g, B, 1],
                                          emxv[0:120, g, 0:WO])
                return emit

            for B in range(NBLK):
                    if B == NBLK - 1:
                        # tail: no pipeline left to protect — let the
                        # previous tree overlap this block's rounds
                        while pending:
                            pending.pop(0)()
                    xb = xblkp.tile([128, BLK_ELEM], f16, tag="x",
                                    name="xb")
                    if B < 2:
                        # ramp: split the first blocks' loads by d-half so
                        # round 0 (d<10) starts before the full block lands
                        half = BLK_ELEM // 2
                        nc.sync.dma_start(xb[:, 0:half], xs[B][:, 0:half])
                        nc.sync.dma_start(xb[:, half:], xs[B][:, half:])
                    else:
                        nc.sync.dma_start(xb[:], xs[B])
                    pvb = xb[:].rearrange("p (d w) -> p d w", w=WP)
                    # both groups' exp layers share one tile so the min
                    # tree runs as joint (2x-wide) DVE ops
                    ebig = expp.tile([128, 2 * 24 * 128], f16, tag="e",
                                     name="ebig")
                    ev = ebig[:].rearrange("p (g l w) -> p g l w",
                                           g=2, w=128)
                    mxj = softp.tile([128, 2 * 128], f16, tag="mx")
                    emxv = mxj[:].rearrange("p (g w) -> p g w", w=128)
                    # 4 rounds; one 4-bank PSUM tile per round holds BOTH
                    # groups (g at 1024-col offsets) so each round drains
                    # with a single FD-2048 ACT/DVE op. Tags alternate u/v
                    # so the PE streams round r+1 while round r drains.
                    for r, chunks in enumerate(ROUNDS4):
                        tagc = "uv"[r % 2]
                        pst = psp.tile([128, 2048], f32, tag=tagc,
                                       name=f"ps{tagc}")
                        for kd in range(3):
                            for c, (d0, dg) in enumerate(chunks):
                                for g in range(NPC):
                                    g64 = 64 * g
                                    base = 1024 * g + 512 * c
                                    ovw = pst[:, base:base + dg * 128] \
                                        .rearrange("p (d w) -> p d w", w=128)
                                    nc.tensor.matmul(
                                        ovw,
                                        lhsT=wt_t[g64:g64 + 64,
                                                  128 * kd:128 * kd + 128],
                                        rhs=pvb[g64:g64 + 64,
                                                d0 + kd:d0 + kd + dg, :],
                                        start=(kd == 0),
                                        stop=(kd == 2),
                                        tile_position=(g64, 0),
                                    )
                        nd = sum(dg for _, dg in chunks)  # 8 or 6
                        if r < 3:
                            # exp(-z) = exp(conv): monotone, so
                            # min_d exp = exp(min_d conv); both groups at once
                            nc.scalar.activation(
                                ev[:, :, EXPL0[r]:EXPL0[r] + nd, :],
                                pst[:].rearrange("p (g d w) -> p g d w",
                                                 g=2, w=128)[:, :, 0:nd, :],
                                ACT_F.Exp, scale=-1.0)
                        else:
                            # direct max(z) over d24-29 from PSUM (joint);
                            # host applies exp(-maxz) and combines
                            nc.vector.tensor_reduce(
                                emxv[:, :, :],
                                pst[:].rearrange("p (g d w) -> p g w d",
                                                 g=2, w=128)[:, :, :, 0:nd],
                                axis=AX.X, op=ALU.max)

                    # lag the tree one block so next block's r3 reduces
                    # release their PSUM tag promptly on the DVE queue
                    while pending:
                        pending.pop(0)()
                    pending.append(tree_block(B, ev, emxv))
            while pending:
                pending.pop(0)()
    nc.compile()
    return nc


def _prep_tables(conv_weight, conv_bias):
    Wn = -np.asarray(conv_weight, np.float32)  # negate: min -> max
    # wt[row k = kw*21 + ci*7 + hl][col = kd*128 + hoff*24 + co]
    wtm = np.zeros((64, 3, 128), np.float32)
    for kw in range(3):
        for ci in range(CI):
            for hl in range(7):
                k = kw * 21 + ci * 7 + hl
                for hoff in range(5):
                    kh = hl - hoff
                    if 0 <= kh < 3:
                        for kd in range(3):
                            wtm[k, kd, hoff * 24 + np.arange(CO)] = \
                                Wn[:, ci, kd, kh, kw]
    wt128 = np.tile(wtm.reshape(64, 3 * 128), (2, 1)).astype(np.float16)
    return wt128


def _block_x(xc):
    """[NPC,3,32,128,128] f32 -> [NBLK, 128, BLK_ELEM] fp16 block tiles.

    Partition g*64 + kw*21 + ci*7 + hl holds, per block B, the w-window
    x[g, ci, :, 5*B+hl, kw:kw+128] zero-padded at the edges.
    """
    xc = np.asarray(xc, np.float16)
    blk = np.zeros((NBLK, 128, D, WP), np.float16)
    for B in range(NBLK):
        for g in range(NPC):
            for kw in range(3):
                wlen = W - kw if kw else W
                for ci in range(CI):
                    for hl in range(7):
                        h = 5 * B + hl
                        if h >= H:
                            continue
                        part = 64 * g + kw * 21 + ci * 7 + hl
                        blk[B, part, :, 0:wlen] = \
                            xc[g, ci, :, h, kw:kw + wlen]
    return blk.reshape(NBLK, 128, BLK_ELEM)


def _get_runner():
    if "runner" in _cache:
        return _cache["runner"]
    import jax
    from jax.experimental.shard_map import shard_map
    from jax.sharding import Mesh, PartitionSpec
    from concourse import bass2jax

    nc = _build_program()
    _cache["nc"] = nc
    bass2jax.install_neuronx_cc_hook()

    import concourse.mybir as mybir

    pname = nc.partition_id_tensor.name if nc.partition_id_tensor else None
    in_names, out_names, out_avals, zero_outs = [], [], [], []
    for alloc in nc.m.functions[0].allocations:
        if not isinstance(alloc, mybir.MemoryLocationSet):
            continue
        name = alloc.memorylocations[0].name
        if alloc.kind == "ExternalInput":
            if name != pname:
                in_names.append(name)
        elif alloc.kind == "ExternalOutput":
            out_names.append(name)
            shape = tuple(alloc.tensor_shape)
            dtype = mybir.dt.np(alloc.dtype)
            out_avals.append(jax.core.ShapedArray(shape, dtype))
            zero_outs.append(np.zeros(shape, dtype))
    n_params = len(in_names)
    n_outs = len(out_avals)
    all_names = in_names + out_names + ([pname] if pname else [])

    def _body(*args):
        operands = list(args)
        if pname:
            operands.append(bass2jax.partition_id_tensor())
        outs = bass2jax._bass_exec_p.bind(
            *operands,
            out_avals=tuple(out_avals),
            in_names=tuple(all_names),
            out_names=tuple(out_names),
            lowering_input_output_aliases=(),
            sim_require_finite=True,
            sim_require_nnan=True,
            nc=nc,
        )
        return tuple(outs)

    devices = jax.devices()[:NCORES]
    mesh = Mesh(np.asarray(devices), ("core",))
    in_specs = (PartitionSpec("core"),) * (n_params + n_outs)
    out_specs = (PartitionSpec("core"),) * n_outs
    donate = tuple(range(n_params, n_params + n_outs))
    sharded = jax.jit(
        shard_map(_body, mesh=mesh, in_specs=in_specs, out_specs=out_specs,
                  check_rep=False),
        donate_argnums=donate, keep_unused=True)

    def run(in_maps):
        per_core = [[np.asarray(m[name]) for name in in_names]
                    for m in in_maps]
        concat_in = [
            np.concatenate([per_core[c][i] for c in range(NCORES)], axis=0)
            for i in range(n_params)
        ]
        concat_zeros = [
            np.zeros((NCORES * z.shape[0], *z.shape[1:]), z.dtype)
            for z in zero_outs
        ]
        out_arrs = sharded(*concat_in, *concat_zeros)
        return [
            {name: np.asarray(out_arrs[i]).reshape(
                NCORES, *out_avals[i].shape)[c]
             for i, name in enumerate(out_names)}
            for c in range(NCORES)
        ]

    _cache["runner"] = run
    return run


def _make_in_maps(x, conv_weight, conv_bias):
    x = np.asarray(x, np.float32)
    wt128 = _prep_tables(conv_weight, conv_bias)
    return [
        {
            "xs": _block_x(x[NPC * c:NPC * (c + 1)]),
            "wt": wt128,
        }
        for c in range(NCORES)
    ]


def kernel(x, conv_weight, conv_bias):
    run = _get_runner()
    in_maps = _make_in_maps(x, conv_weight, conv_bias)
    results = run(in_maps)
    outs = [results[c]["eout"] for c in range(NCORES)]
    eo = np.concatenate(outs, axis=0).astype(np.float32)  # [16,26,2,120,126]
    e = np.minimum(eo[:, :, 0], np.exp(-eo[:, :, 1]))
    # [n, B, hoff, co, w] -> [n, co, 5B+hoff, w]
    e = e.reshape(N_TOT, NBLK, 5, CO, WO).transpose(0, 3, 1, 2, 4)
    e = e.reshape(N_TOT, CO, NBLK * 5, WO)[:, :, 0:HO, :]
    num = e * np.exp(np.asarray(conv_bias, np.float64))[None, :, None, None]
    out = num / num.sum(axis=1, keepdims=True)
    return out.reshape(N_TOT, CO, 1, HO, WO).astype(np.float32)


# revision 18
# speedup vs baseline: 1.6189x; 1.6189x over previous
"""Trainium2 Bass kernel v4: Conv3d(3,24,k=3,VALID) -> min depth -> softmax(ch).

Input  x: [16,3,32,128,128] f32, conv_weight [24,3,3,3,3], conv_bias [24].
Output: [16,24,1,126,126] f32.

v4 strategy (per core; batch-sharded 2 samples/core over 8 cores):
 - kw folded into K: partition k = g*64 + kw*21 + ci*7 + hl (63 used rows per
   64-row group g = sample). Host pre-shifts each kw copy's w-window so the
   matmul rhs is one contiguous [dg,128] run. Only kd remains as 3 PSUM
   accumulation passes (vs 9 in v3) -> 3x fewer streamed columns.
 - 2 concurrent 64-row PE tile groups (one per sample), M = 120 = hoff*24+co.
 - 26 h-blocks of 5 output rows per sample; per block 4 rounds of 2 PSUM
   banks, chunks (4d,128w)x7 + (2d,128w)x1 = 30 depth layers.
 - Epilogue: monotone-exp trick: exp(min y) = min exp(y). ACT streams most
   PSUM banks through exp(-z+bias)->fp16 SBUF; DVE direct-max-reduces the
   rest from PSUM; DVE fp16 TT-min tree (2x mode) collapses depth.
 - Softmax: PE ones-matmul sums channels, DVE reciprocal, PE broadcast
   matmul, DVE multiply. No 128x128 transpose.
"""
import sys

sys.path.insert(0, "/opt/trn_rl_repo")

import numpy as np

# Problem constants
N_TOT, CI, D, H, W = 16, 3, 32, 128, 128
CO = 24
DO, HO, WO = 30, 126, 126
NCORES = 8
NPC = N_TOT // NCORES  # samples per core = 2

NBLK = 26            # h-blocks of 5 output rows
XBUFS = 10           # per-block input tiles in the prefetch rotation
WP = 128             # stored w extent per partition (pre-shifted per kw)
BLK_ELEM = D * WP    # 4096 elements per (partition, block)
# four 4-bank PSUM rounds per block: depth chunks (d0, dg).
# Rounds 0-2 drain via ACT exp; round 3 via DVE direct max-reduce.
ROUNDS4 = [((0, 4), (4, 4)), ((8, 4), (12, 4)),
           ((16, 4), (20, 4)), ((24, 4), (28, 2))]
EXPL0 = (0, 8, 16)   # eblk layer base per exp round

_cache = {}


def _build_program():
    import concourse.bass as bass
    import concourse.mybir as mybir
    from concourse import bacc, tile

    dt = mybir.dt
    f32 = dt.float32
    f16 = dt.float16
    AX = mybir.AxisListType
    ALU = mybir.AluOpType
    ACT_F = mybir.ActivationFunctionType

    nc = bacc.Bacc("TRN2", target_bir_lowering=False, debug=False)

    xs = nc.dram_tensor("xs", [NBLK, 128, BLK_ELEM], f16,
                        kind="ExternalInput")
    wt = nc.dram_tensor("wt", [128, 3 * 128], f16, kind="ExternalInput")
    # per block: [kind 0] min_{d<24} exp(conv) and [kind 1] max_{d>=24} z;
    # host combines as min(e, exp(-maxz)) then softmaxes. [n,block,kind,p,w]
    eout = nc.dram_tensor("eout", [NPC, NBLK, 2, 120, WO], f16,
                          kind="ExternalOutput")

    with tile.TileContext(nc) as tc:
        with (
            tc.tile_pool(name="const", bufs=1) as constp,
            tc.tile_pool(name="xblk", bufs=XBUFS) as xblkp,
            tc.tile_pool(name="expp", bufs=3) as expp,
            tc.tile_pool(name="soft", bufs=3) as softp,
            tc.tile_pool(name="ps", bufs=1, space="PSUM") as psp,
        ):
            wt_t = constp.tile([128, 3 * 128], f16)
            nc.sync.dma_start(wt_t[:], wt[:])

            pending = []  # deferred tree closures (lagged one block)

            def tree_block(B, ev, emxv):
                def emit():
                    # joint min tree over 24 exp layers for both groups
                    s1 = softp.tile([128, 2 * 12 * 128], f16, tag="s1")
                    s1v = s1[:].rearrange("p (g l w) -> p g l w",
                                          g=2, w=128)
                    nc.vector.tensor_tensor(
                        s1v[:, :, 0:12, :], ev[:, :, 0:12, :],
                        ev[:, :, 12:24, :], op=ALU.min)
                    s2 = softp.tile([128, 2 * 6 * 128], f16, tag="s2")
                    s2v = s2[:].rearrange("p (g l w) -> p g l w",
                                          g=2, w=128)
                    nc.vector.tensor_tensor(
                        s2v[:, :, 0:6, :], s1v[:, :, 0:6, :],
                        s1v[:, :, 6:12, :], op=ALU.min)
                    s3 = softp.tile([128, 2 * 3 * 128], f16, tag="s3")
                    s3v = s3[:].rearrange("p (g l w) -> p g l w",
                                          g=2, w=128)
                    nc.vector.tensor_tensor(
                        s3v[:, :, 0:3, :], s2v[:, :, 0:3, :],
                        s2v[:, :, 3:6, :], op=ALU.min)
                    s4 = softp.tile([128, 2 * 2 * 128], f16, tag="s4")
                    s4v = s4[:].rearrange("p (g l w) -> p g l w",
                                          g=2, w=128)
                    nc.vector.tensor_tensor(
                        s4v[:, :, 0:2, :], s3v[:, :, 0:2, :],
                        s3v[:, :, 1:3, :], op=ALU.min)
                    e16 = softp.tile([128, 2 * 128], f16, tag="e16")
                    e16v = e16[:].rearrange("p (g w) -> p g w", w=128)
                    nc.vector.tensor_tensor(
                        e16v[:, :, :], s4v[:, :, 0, :], s4v[:, :, 1, :],
                        op=ALU.min)
                    for g in range(NPC):
                        nc.sync.dma_start(eout[g, B, 0],
                                          e16v[0:120, g, 0:WO])
                        nc.sync.dma_start(eout[g, B, 1],
                                          emxv[0:120, g, 0:WO])
                return emit

            for B in range(NBLK):
                    if B == NBLK - 1:
                        # tail: no pipeline left to protect — let the
                        # previous tree overlap this block's rounds
                        while pending:
                            pending.pop(0)()
                    xb = xblkp.tile([128, BLK_ELEM], f16, tag="x",
                                    name="xb")
                    if B < 2:
                        # ramp: split the first blocks' loads by d-half so
                        # round 0 (d<10) starts before the full block lands
                        half = BLK_ELEM // 2
                        nc.sync.dma_start(xb[:, 0:half], xs[B][:, 0:half])
                        nc.sync.dma_start(xb[:, half:], xs[B][:, half:])
                    else:
                        nc.sync.dma_start(xb[:], xs[B])
                    pvb = xb[:].rearrange("p (d w) -> p d w", w=WP)
                    # both groups' exp layers share one tile so the min
                    # tree runs as joint (2x-wide) DVE ops
                    ebig = expp.tile([128, 2 * 24 * 128], f16, tag="e",
                                     name="ebig")
                    ev = ebig[:].rearrange("p (g l w) -> p g l w",
                                           g=2, w=128)
                    mxj = softp.tile([128, 2 * 128], f16, tag="mx")
                    emxv = mxj[:].rearrange("p (g w) -> p g w", w=128)
                    # 4 rounds; one 4-bank PSUM tile per round holds BOTH
                    # groups (g at 1024-col offsets) so each round drains
                    # with a single FD-2048 ACT/DVE op. Tags alternate u/v
                    # so the PE streams round r+1 while round r drains.
                    for r, chunks in enumerate(ROUNDS4):
                        tagc = "uv"[r % 2]
                        pst = psp.tile([128, 2048], f32, tag=tagc,
                                       name=f"ps{tagc}")
                        for kd in range(3):
                            for c, (d0, dg) in enumerate(chunks):
                                for g in range(NPC):
                                    g64 = 64 * g
                                    base = 1024 * g + 512 * c
                                    ovw = pst[:, base:base + dg * 128] \
                                        .rearrange("p (d w) -> p d w", w=128)
                                    nc.tensor.matmul(
                                        ovw,
                                        lhsT=wt_t[g64:g64 + 64,
                                                  128 * kd:128 * kd + 128],
                                        rhs=pvb[g64:g64 + 64,
                                                d0 + kd:d0 + kd + dg, :],
                                        start=(kd == 0),
                                        stop=(kd == 2),
                                        tile_position=(g64, 0),
                                    )
                        nd = sum(dg for _, dg in chunks)  # 8 or 6
                        if r < 3:
                            # exp(-z) = exp(conv): monotone, so
                            # min_d exp = exp(min_d conv); both groups at once
                            nc.scalar.activation(
                                ev[:, :, EXPL0[r]:EXPL0[r] + nd, :],
                                pst[:].rearrange("p (g d w) -> p g d w",
                                                 g=2, w=128)[:, :, 0:nd, :],
                                ACT_F.Exp, scale=-1.0)
                        else:
                            # direct max(z) over d24-29 from PSUM (joint);
                            # host applies exp(-maxz) and combines
                            nc.vector.tensor_reduce(
                                emxv[:, :, :],
                                pst[:].rearrange("p (g d w) -> p g w d",
                                                 g=2, w=128)[:, :, :, 0:nd],
                                axis=AX.X, op=ALU.max)

                    # lag the tree one block so next block's r3 reduces
                    # release their PSUM tag promptly on the DVE queue
                    while pending:
                        pending.pop(0)()
                    pending.append(tree_block(B, ev, emxv))
            while pending:
                pending.pop(0)()
    nc.compile()
    return nc


def _prep_tables(conv_weight, conv_bias):
    Wn = -np.asarray(conv_weight, np.float32)  # negate: min -> max
    # wt[row k = kw*21 + ci*7 + hl][col = kd*128 + hoff*24 + co]
    wtm = np.zeros((64, 3, 128), np.float32)
    for kw in range(3):
        for ci in range(CI):
            for hl in range(7):
                k = kw * 21 + ci * 7 + hl
                for hoff in range(5):
                    kh = hl - hoff
                    if 0 <= kh < 3:
                        for kd in range(3):
                            wtm[k, kd, hoff * 24 + np.arange(CO)] = \
                                Wn[:, ci, kd, kh, kw]
    wt128 = np.tile(wtm.reshape(64, 3 * 128), (2, 1)).astype(np.float16)
    return wt128


def _block_x(xc):
    """[NPC,3,32,128,128] f32 -> [NBLK, 128, BLK_ELEM] fp16 block tiles.

    Partition g*64 + kw*21 + ci*7 + hl holds, per block B, the w-window
    x[g, ci, :, 5*B+hl, kw:kw+128] zero-padded at the edges.
    """
    xc = np.asarray(xc, np.float16)
    blk = np.zeros((NBLK, 128, D, WP), np.float16)
    for B in range(NBLK):
        for g in range(NPC):
            for kw in range(3):
                wlen = W - kw if kw else W
                for ci in range(CI):
                    for hl in range(7):
                        h = 5 * B + hl
                        if h >= H:
                            continue
                        part = 64 * g + kw * 21 + ci * 7 + hl
                        blk[B, part, :, 0:wlen] = \
                            xc[g, ci, :, h, kw:kw + wlen]
    return blk.reshape(NBLK, 128, BLK_ELEM)


def _get_runner():
    if "runner" in _cache:
        return _cache["runner"]
    import jax
    from jax.experimental.shard_map import shard_map
    from jax.sharding import Mesh, PartitionSpec
    from concourse import bass2jax

    nc = _build_program()
    _cache["nc"] = nc
    bass2jax.install_neuronx_cc_hook()

    import concourse.mybir as mybir

    pname = nc.partition_id_tensor.name if nc.partition_id_tensor else None
    in_names, out_names, out_avals, zero_outs = [], [], [], []
    for alloc in nc.m.functions[0].allocations:
        if not isinstance(alloc, mybir.MemoryLocationSet):
            continue
        name = alloc.memorylocations[0].name
        if alloc.kind == "ExternalInput":
            if name != pname:
                in_names.append(name)
        elif alloc.kind == "ExternalOutput":
            out_names.append(name)
            shape = tuple(alloc.tensor_shape)
            dtype = mybir.dt.np(alloc.dtype)
            out_avals.append(jax.core.ShapedArray(shape, dtype))
            zero_outs.append(np.zeros(shape, dtype))
    n_params = len(in_names)
    n_outs = len(out_avals)
    all_names = in_names + out_names + ([pname] if pname else [])

    def _body(*args):
        operands = list(args)
        if pname:
            operands.append(bass2jax.partition_id_tensor())
        outs = bass2jax._bass_exec_p.bind(
            *operands,
            out_avals=tuple(out_avals),
            in_names=tuple(all_names),
            out_names=tuple(out_names),
            lowering_input_output_aliases=(),
            sim_require_finite=True,
            sim_require_nnan=True,
            nc=nc,
        )
        return tuple(outs)

    devices = jax.devices()[:NCORES]
    mesh = Mesh(np.asarray(devices), ("core",))
    in_specs = (PartitionSpec("core"),) * (n_params + n_outs)
    out_specs = (PartitionSpec("core"),) * n_outs
    donate = tuple(range(n_params, n_params + n_outs))
    sharded = jax.jit(
        shard_map(_body, mesh=mesh, in_specs=in_specs, out_specs=out_specs,
                  check_rep=False),
        donate_argnums=donate, keep_unused=True)

    def run(in_maps):
        per_core = [[np.asarray(m[name]) for name in in_names]
                    for m in in_maps]
        concat_in = [
            np.concatenate([per_core[c][i] for c in range(NCORES)], axis=0)
            for i in range(n_params)
        ]
        concat_zeros = [
            np.zeros((NCORES * z.shape[0], *z.shape[1:]), z.dtype)
            for z in zero_outs
        ]
        out_arrs = sharded(*concat_in, *concat_zeros)
        return [
            {name: np.asarray(out_arrs[i]).reshape(
                NCORES, *out_avals[i].shape)[c]
             for i, name in enumerate(out_names)}
            for c in range(NCORES)
        ]

    _cache["runner"] = run
    return run


def _make_in_maps(x, conv_weight, conv_bias):
    x = np.asarray(x, np.float32)
    wt128 = _prep_tables(conv_weight, conv_bias)
    return [
        {
            "xs": _block_x(x[NPC * c:NPC * (c + 1)]),
            "wt": wt128,
        }
        for c in range(NCORES)
    ]


def kernel(x, conv_weight, conv_bias):
    run = _get_runner()
    in_maps = _make_in_maps(x, conv_weight, conv_bias)
    results = run(in_maps)
    outs = [results[c]["eout"] for c in range(NCORES)]
    eo = np.concatenate(outs, axis=0).astype(np.float32)  # [16,26,2,120,126]
    e = np.minimum(eo[:, :, 0], np.exp(-eo[:, :, 1]))
    # [n, B, hoff, co, w] -> [n, co, 5B+hoff, w]
    e = e.reshape(N_TOT, NBLK, 5, CO, WO).transpose(0, 3, 1, 2, 4)
    e = e.reshape(N_TOT, CO, NBLK * 5, WO)[:, :, 0:HO, :]
    num = e * np.exp(np.asarray(conv_bias, np.float64))[None, :, None, None]
    out = num / num.sum(axis=1, keepdims=True)
    return out.reshape(N_TOT, CO, 1, HO, WO).astype(np.float32)
